# revision 57
# baseline (speedup 1.0000x reference)
"""Trainium2 Bass kernel for nn_EncoderBlock (dense transformer encoder block).

Strategy (8 NeuronCores, head-parallel attention):
  - Cores 0-3 handle batch 0, cores 4-7 batch 1. Within a group each core
    owns 4 heads for attention (all 2048 tokens) and 512 tokens for the FFN.
  - Each core loads the full batch x^T, computes LN1 for all 2048 tokens
    (duplicated across the group - cheap vector work, zero comms), then
    projects Q/K/V for its 4 heads only (weight columns sliced host-side).
    Attention runs on transposed scores [k,q] with the softmax sum folded
    into the AV matmul via a ones column in V; no max subtraction needed.
  - W_o partial products (contraction over this core's 256 ctx features)
    are exchanged with a single ReduceScatter (2MB output) - the only
    collective in the kernel - yielding each core's own 512-token slice.
  - Residual, LN2, FFN token-parallel with full W1/W2 in bf16 (halves the
    32MB weight DMA; same PE rate as f32r).
  - Matmuls in float32r (TF32): 1 cycle/row on the PE at N>=256.
"""

import numpy as np

import ml_dtypes

import concourse.bass as bass
import concourse.mybir as mybir
import concourse.tile as tile
from concourse import bacc
from concourse.bass_utils import run_bass_kernel_spmd

N_CORES = 8
GRP = 4          # cores per batch group
P = 128
TOK = 512        # own tokens per core (FFN / output slice)
S = 2048         # sequence length (tokens per batch)
D = 1024
KT = D // P      # 8 feature tiles
HF = 256         # head features per core (4 heads x 64)
NPAIR = 2        # head pairs per core
DK = 64
DK1 = DK + 1
VW = 128       # AV block: [ones@0 | m*r@32 | V@64:128]
F = 4096
FT = F // P      # 32 ffn tiles
EPS = 1e-6
SCALE = 0.125    # 1/sqrt(DK)
MT_S = S // P    # 16 k-token tiles per batch
QC = S // TOK    # 4 query chunks of 512
NBLK = 2         # score blocks per psum tile / exp call

f32 = mybir.dt.float32
f32r = mybir.dt.float32r
bf16 = mybir.dt.bfloat16
ALU = mybir.AluOpType
ACT = mybir.ActivationFunctionType


def tf32_round(x: np.ndarray) -> np.ndarray:
    u = np.ascontiguousarray(x, dtype=np.float32).view(np.uint32)
    lsb = (u >> np.uint32(13)) & np.uint32(1)
    u = u + np.uint32(0x0FFF) + lsb
    u = u & np.uint32(0xFFFFE000)
    return u.view(np.float32)


def _layer_norm(nc, tc, hpool, ones_b, x_tiles, g_t, b_t, tag, width,
                in_bf16=False, inplace=False):
    """Feature-major layernorm over KT [128, width] tiles, bf16 output.

    Per-token (free-dim) stats via ones-matmul partition reduction on PE,
    chunked by 512 columns (PSUM bank limit). r = 1/sqrt on ACT.Sqrt + DVE
    reciprocal (avoids Ln/Exp table swaps). If inplace, the normalized
    output overwrites x_tiles (requires in_bf16).
    """
    CH = width // 512
    rows = tc.alloc_tile_pool(name=f"lnrow_{tag}", bufs=1)
    ltr = tc.alloc_tile_pool(name=f"lntr_{tag}", bufs=2)
    lnps = tc.alloc_tile_pool(name=f"lnps_{tag}", bufs=2, space="PSUM")

    inv_n = 1.0 / D
    h_tiles = []
    if not inplace:
        for kt in range(KT):
            h_tiles.append(hpool.tile([P, width], bf16,
                                      name=f"h_{tag}_{kt}", tag=f"h_{kt}"))
    mean = rows.tile([1, width], f32, name=f"mean_{tag}", tag="mean")
    var = rows.tile([1, width], f32, name=f"var_{tag}", tag="var")
    for ch in range(CH):
        cs = slice(ch * 512, (ch + 1) * 512)
        ps_sum = lnps.tile([1, 512], f32, name=f"pssum_{tag}_{ch}", tag="pssum")
        ps_sq = lnps.tile([1, 512], f32, name=f"pssq_{tag}_{ch}", tag="pssq")
        for kt in range(KT):
            sq = ltr.tile([P, 512], bf16, name=f"sq_{tag}_{ch}_{kt}",
                          tag="xrsq", bufs=3)
            nc.vector.tensor_mul(sq[:], x_tiles[kt][:, cs], x_tiles[kt][:, cs])
            if in_bf16:
                xr_ap = x_tiles[kt][:, cs]
            else:
                xr = ltr.tile([P, 512], bf16, name=f"xr_{tag}_{ch}_{kt}",
                              tag="xrsq", bufs=3)
                nc.scalar.copy(xr[:], x_tiles[kt][:, cs])
                xr_ap = xr[:]
            nc.tensor.matmul(ps_sum[:], lhsT=ones_b[:], rhs=xr_ap,
                             start=(kt == 0), stop=(kt == KT - 1))
            nc.tensor.matmul(ps_sq[:], lhsT=ones_b[:], rhs=sq[:],
                             start=(kt == 0), stop=(kt == KT - 1))
        msq = rows.tile([1, 512], f32, name=f"msq_{tag}_{ch}", tag="msqlnv",
                        bufs=2)
        nc.vector.tensor_scalar_mul(mean[:, cs], ps_sum[:], inv_n)
        nc.vector.tensor_scalar_mul(var[:, cs], ps_sq[:], inv_n)
        nc.vector.tensor_mul(msq[:], mean[:, cs], mean[:, cs])
        nc.vector.tensor_sub(var[:, cs], var[:, cs], msq[:])
        nc.vector.tensor_scalar_add(var[:, cs], var[:, cs], EPS)

    # r = 1/sqrt(var+eps): ACT.Sqrt then DVE reciprocal
    sd = rows.tile([1, width], f32, name=f"sd_{tag}", tag="sd")
    nc.scalar.activation(sd[:], var[:], ACT.Sqrt)
    r_row = rows.tile([1, width], bf16, name=f"r_{tag}", tag="r")
    with nc.allow_low_precision(reason="bf16 layernorm scale is plenty"):
        nc.vector.reciprocal(r_row[:], sd[:])
    mr_row = rows.tile([1, width], bf16, name=f"mr_{tag}", tag="mr")
    nc.vector.tensor_mul(mr_row[:], mean[:], r_row[:])

    for ch in range(CH):
        cs = slice(ch * 512, (ch + 1) * 512)
        r_bc = rows.tile([P, 512], bf16, name=f"rbc_{tag}_{ch}", tag="rbc",
                         bufs=2)
        mr_bc = rows.tile([P, 512], bf16, name=f"mrbc_{tag}_{ch}", tag="mrbc",
                          bufs=2)
        nc.gpsimd.partition_broadcast(r_bc[:], r_row[:, cs])
        nc.gpsimd.partition_broadcast(mr_bc[:], mr_row[:, cs])
        for kt in range(KT):
            t1 = ltr.tile([P, 512], bf16, name=f"t1_{tag}_{ch}_{kt}",
                          tag="lnt1", bufs=3)
            nc.vector.tensor_mul(t1[:], x_tiles[kt][:, cs], r_bc[:])
            nc.vector.tensor_sub(t1[:], t1[:], mr_bc[:])
            h = x_tiles[kt] if inplace else h_tiles[kt]
            nc.vector.tensor_scalar(h[:, cs], t1[:], g_t[:, kt:kt + 1],
                                    b_t[:, kt:kt + 1], ALU.mult, ALU.add)
    if inplace:
        h_tiles = x_tiles
    lnps.release()
    ltr.release()
    rows.release()
    return h_tiles


def build(n_iters: int = 1):
    nc = bacc.Bacc("TRN2", target_bir_lowering=False, debug=False,
                   num_devices=N_CORES)

    xT = nc.dram_tensor("xT", [D, S], bf16, kind="ExternalInput").ap()
    xsT = nc.dram_tensor("xsT", [D, TOK], f32, kind="ExternalInput").ap()
    wqh = nc.dram_tensor("wqh", [D, HF], bf16, kind="ExternalInput").ap()
    wkh = nc.dram_tensor("wkh", [D, HF], bf16, kind="ExternalInput").ap()
    wvh = nc.dram_tensor("wvh", [D, HF], bf16, kind="ExternalInput").ap()
    woh = nc.dram_tensor("woh", [HF, D], bf16, kind="ExternalInput").ap()
    w1 = nc.dram_tensor("w1", [D, F], bf16, kind="ExternalInput").ap()
    w2 = nc.dram_tensor("w2", [F, D], bf16, kind="ExternalInput").ap()
    aux_v = nc.dram_tensor("aux_v", [P, 76], f32, kind="ExternalInput").ap()

    outT = nc.dram_tensor("outT", [D, TOK], f32, kind="ExternalOutput").ap()

    groups = [[0, 1, 2, 3], [4, 5, 6, 7]]

    with tile.TileContext(nc) as tc:
        sb = tc.alloc_tile_pool(name="sb", bufs=1)
        tr = tc.alloc_tile_pool(name="tr", bufs=3)
        wp = tc.alloc_tile_pool(name="wp", bufs=2)
        ep = tc.alloc_tile_pool(name="ep", bufs=3)
        dram = tc.alloc_tile_pool(name="dram", bufs=1, space="DRAM")

        # ---- constants / small inputs ----
        ones_f = sb.tile([P, 32], bf16, name="ones_f", tag="ones_f")
        nc.vector.memset(ones_f[:], 1.0)
        ones_b = sb.tile([P, 1], bf16, name="ones_b", tag="ones_b")
        nc.vector.tensor_copy(ones_b[:], ones_f[:, 0:1])
        ones_g = sb.tile([P, 1], f32, name="ones_g", tag="ones_g")
        nc.vector.memset(ones_g[:], 1.0)
        aux_t = sb.tile([P, 76], f32, name="aux_t", tag="aux_t")
        nc.sync.dma_start(out=aux_t[:], in_=aux_v)
        bo_t = aux_t[:, 0:8]
        b1_t = aux_t[:, 8:40]
        b2_t = aux_t[:, 40:48]
        g2_t = aux_t[:, 48:56]
        be2_t = aux_t[:, 56:64]

        part = dram.tile([GRP * D, TOK], bf16, name="part", tag="part")
        rsout = dram.tile([D, TOK], bf16, name="rsout", tag="rsout")
        rrow_d = dram.tile([1, S], f32, name="rrow_d", tag="rrow_d")
        mrrow_d = dram.tile([1, S], f32, name="mrrow_d", tag="mrrow_d")

        for it in range(n_iters):
            # ---- load QKV weight slices (full x comes last in the stack) ----
            qkvw = tc.alloc_tile_pool(name=f"qkvw{it}", bufs=1)
            wgt = {}
            for w_dram, nm in [(wkh, "k"), (wvh, "v"), (wqh, "q")]:
                t = qkvw.tile([P, KT * HF], bf16, name=f"w{nm}g_{it}",
                              tag=f"w{nm}g")
                nc.sync.dma_start(
                    out=t[:].rearrange("p (k c) -> p k c", k=KT),
                    in_=w_dram.rearrange("(k p) c -> p k c", p=P))
                wgt[nm] = t
            wog = qkvw.tile([P, NPAIR * D], bf16, name=f"wog_{it}", tag="wog")
            nc.sync.dma_start(
                out=wog[:].rearrange("p (k c) -> p k c", k=NPAIR),
                in_=woh.rearrange("(k p) c -> p k c", p=P))

            # attention destination pools sit below xp so xp can release first
            kqp = tc.alloc_tile_pool(name=f"kqp{it}", bufs=1)
            k_tiles, q_tiles = [], []
            for nm, lst in [("k", k_tiles), ("q", q_tiles)]:
                for pt in range(NPAIR):
                    lst.append(kqp.tile([P, S], bf16, name=f"{nm}T_{it}_{pt}",
                                        tag=f"{nm}T_{pt}"))
            vpp = tc.alloc_tile_pool(name=f"vpp{it}", bufs=1)
            vp_tiles = []
            for pt in range(NPAIR):
                vp = vpp.tile([P, MT_S * 2 * VW], bf16, name=f"vp_{it}_{pt}",
                              tag=f"vp_{pt}")
                nc.vector.memset(vp[:], 0.0)
                vp_tiles.append(vp)
            cp = tc.alloc_tile_pool(name=f"cp{it}", bufs=1)
            ctx_tiles = []
            for pt in range(NPAIR):
                ctx_tiles.append(cp.tile([P, S], bf16, name=f"ctx_{it}_{pt}",
                                         tag=f"ctx_{pt}"))

            xp = tc.alloc_tile_pool(name=f"xp{it}", bufs=1)
            xg = xp.tile([P, KT * S], bf16, name=f"xg_{it}", tag="xg")
            x_tiles = [xg[:, kt * S:(kt + 1) * S] for kt in range(KT)]
            # two column-chunked loads so LN1 stats can start early
            for ch in range(2):
                cs = slice(ch * 1024, (ch + 1) * 1024)
                nc.sync.dma_start(
                    out=xg[:].rearrange("p (k c) -> p k c", k=KT)[:, :, cs],
                    in_=xT[:, cs].rearrange("(k p) c -> p k c", p=P))

            # ---- LN1 stats on raw x (projections absorb the normalization:
            #      q = r*(Wq'^T x - m*wbar_q) + qb, with Wq' = diag(g)Wq and
            #      wbar/qb precomputed host-side) ----
            lnp = tc.alloc_tile_pool(name=f"lnp{it}", bufs=1)
            ltr = tc.alloc_tile_pool(name=f"lntr{it}", bufs=2)
            lnps = tc.alloc_tile_pool(name=f"lnps{it}", bufs=2, space="PSUM")
            mean = lnp.tile([1, S], f32, name=f"mean_{it}", tag="mean")
            var = lnp.tile([1, S], f32, name=f"var_{it}", tag="var")
            for ch in range(QC):
                cs = slice(ch * 512, (ch + 1) * 512)
                ps_sum = lnps.tile([1, 512], f32, name=f"pssum_{ch}", tag="pssum")
                ps_sq = lnps.tile([1, 512], f32, name=f"pssq_{ch}", tag="pssq")
                for kt in range(KT):
                    sq = ltr.tile([P, 512], bf16, name=f"sq_{ch}_{kt}",
                                  tag="xrsq", bufs=3)
                    nc.scalar.square(sq[:], x_tiles[kt][:, cs])
                    nc.tensor.matmul(ps_sum[:], lhsT=ones_b[:],
                                     rhs=x_tiles[kt][:, cs],
                                     start=(kt == 0), stop=(kt == KT - 1))
                    nc.tensor.matmul(ps_sq[:], lhsT=ones_b[:], rhs=sq[:],
                                     start=(kt == 0), stop=(kt == KT - 1))
                msq = lnp.tile([1, 512], f32, name=f"msq_{ch}", tag="msq",
                               bufs=1)
                nc.vector.tensor_scalar_mul(mean[:, cs], ps_sum[:], 1.0 / D)
                nc.vector.tensor_scalar_mul(var[:, cs], ps_sq[:], 1.0 / D)
                nc.vector.tensor_mul(msq[:], mean[:, cs], mean[:, cs])
                nc.vector.tensor_sub(var[:, cs], var[:, cs], msq[:])
                nc.vector.tensor_scalar_add(var[:, cs], var[:, cs], EPS)
            m_row = lnp.tile([1, S], bf16, name=f"m_{it}", tag="m")
            nc.vector.tensor_copy(m_row[:], mean[:])
            # var -> sqrt -> reciprocal in place; mean -> mean*r in place
            nc.scalar.activation(var[:], var[:], ACT.Sqrt)
            nc.vector.reciprocal(var[:], var[:])
            r_row = lnp.tile([1, S], bf16, name=f"r_{it}", tag="r")
            nc.vector.tensor_copy(r_row[:], var[:])
            nc.vector.tensor_mul(mean[:], mean[:], var[:])
            # token-major r and m*r columns for the V path
            rcol = lnp.tile([P, MT_S], f32, name=f"rcol_{it}", tag="rcol")
            mrcol = lnp.tile([P, MT_S], f32, name=f"mrcol_{it}", tag="mrcol")
            nc.sync.dma_start(out=rrow_d[:], in_=var[:])
            nc.sync.dma_start(out=mrrow_d[:], in_=mean[:])
            nc.sync.dma_start(
                out=rcol[:], in_=rrow_d[:].rearrange("o (m p) -> (o p) m", p=P))
            nc.sync.dma_start(
                out=mrcol[:],
                in_=mrrow_d[:].rearrange("o (m p) -> (o p) m", p=P))
            lnps.release()

            # ---- K^T / Q^T projections on raw x + LN correction ----
            qkvps = tc.alloc_tile_pool(name=f"qkvps{it}", bufs=4, space="PSUM")
            mbc_tiles, rbc_tiles = [], []
            for ch in range(QC):
                cs = slice(ch * 512, (ch + 1) * 512)
                m_bc = lnp.tile([P, 512], bf16, name=f"mbc_{ch}", tag=f"mbc_{ch}")
                r_bc = lnp.tile([P, 512], bf16, name=f"rbc_{ch}", tag=f"rbc_{ch}")
                nc.gpsimd.partition_broadcast(m_bc[:], m_row[:, cs])
                nc.gpsimd.partition_broadcast(r_bc[:], r_row[:, cs])
                mbc_tiles.append(m_bc)
                rbc_tiles.append(r_bc)
            for dsts, nm, wb_i, bb_i in [
                    (k_tiles, "k", 2, 2), (q_tiles, "q", 0, 0)]:
                wg = wgt[nm]
                for pt in range(NPAIR):
                    dst = dsts[pt]
                    for ch in range(QC):
                        cs = slice(ch * 512, (ch + 1) * 512)
                        ps = qkvps.tile([P, 512], f32, name=f"ps{nm}_{pt}_{ch}",
                                        tag="qkv")
                        for kt in range(KT):
                            nc.tensor.matmul(
                                ps[:],
                                lhsT=wg[:, kt * HF + pt * P:kt * HF + (pt + 1) * P],
                                rhs=x_tiles[kt][:, cs], start=(kt == 0),
                                stop=(kt == KT - 1))
                        u = ltr.tile([P, 512], bf16, name=f"u{nm}_{pt}_{ch}",
                                     tag="u", bufs=3)
                        nc.vector.scalar_tensor_tensor(
                            u[:], mbc_tiles[ch][:],
                            aux_t[:, 64 + wb_i + pt:65 + wb_i + pt], ps[:],
                            ALU.mult, ALU.add)
                        nc.vector.tensor_mul(dst[:, cs], u[:], rbc_tiles[ch][:])
                        nc.vector.tensor_scalar_add(
                            dst[:, cs], dst[:, cs],
                            aux_t[:, 68 + bb_i + pt:69 + bb_i + pt])

            # ---- V projection (token-major, [r*V | m*r | 1] packing) ----
            for pt in range(NPAIR):
                vpv = vp_tiles[pt][:].rearrange("q (k c) -> q k c", c=VW)
                nc.vector.tensor_copy(vpv[:, :, 0:1].squeeze(2), ones_f[:])
                for hf in range(2):
                    nc.vector.tensor_copy(
                        vp_tiles[pt][:].rearrange("q (m h c) -> q m h c", h=2,
                                                  c=VW)
                        [:, :, hf, 32:33].squeeze(2), mrcol[:])
            for mt in range(MT_S):
                ps = qkvps.tile([P, HF], f32, name=f"psv_{mt}", tag="qkv")
                for kt in range(KT):
                    nc.tensor.matmul(
                        ps[:], lhsT=x_tiles[kt][:, mt * P:(mt + 1) * P],
                        rhs=wgt["v"][:, kt * HF:(kt + 1) * HF],
                        start=(kt == 0), stop=(kt == KT - 1))
                for pt in range(NPAIR):
                    for hf in range(2):
                        dst = vp_tiles[pt][:].rearrange(
                            "q (m c) -> q m c", c=2 * VW)[:, mt,
                            hf * VW + DK:hf * VW + P]
                        nc.vector.tensor_scalar_mul(
                            dst, ps[:, pt * P + hf * DK:pt * P + (hf + 1) * DK],
                            rcol[:, mt:mt + 1])
            qkvps.release()
            ltr.release()
            lnp.release()
            xp.release()


            # ---- attention (4 query chunks x 2 head pairs), W_o fused ----
            scps = tc.alloc_tile_pool(name=f"scps{it}", bufs=2, space="PSUM")
            ctxps = tc.alloc_tile_pool(name=f"ctxps{it}", bufs=2, space="PSUM")
            wops = tc.alloc_tile_pool(name=f"wops{it}", bufs=2, space="PSUM")
            for ch in range(QC):
                cs = slice(ch * 512, (ch + 1) * 512)
                for pt in range(NPAIR):
                    ps_ctx = [ctxps.tile([P, 512], f32,
                                         name=f"psctx_{pt}_{ch}_{hh}",
                                         tag="psctx") for hh in range(2)]
                    ps_sc = None
                    e_t = None
                    nb = 0
                    for i in range(2 * MT_S):
                        mt, half = i >> 1, i & 1
                        j = i % NBLK
                        if j == 0:
                            nb = min(NBLK, 2 * MT_S - i)
                            ps_sc = scps.tile([P, NBLK * 512], f32,
                                              name=f"pssc_{pt}_{ch}_{i}",
                                              tag="pssc")
                            e_t = ep.tile([P, NBLK * 512], bf16,
                                          name=f"e_{pt}_{ch}_{i}", tag="e")
                        nc.tensor.matmul(
                            ps_sc[:, j * 512:(j + 1) * 512],
                            lhsT=k_tiles[pt][half * DK:(half + 1) * DK,
                                             mt * P:(mt + 1) * P],
                            rhs=q_tiles[pt][half * DK:(half + 1) * DK, cs],
                            start=True, stop=True)
                        if j == nb - 1:
                            nc.scalar.activation(e_t[:, 0:nb * 512],
                                                 ps_sc[:, 0:nb * 512],
                                                 ACT.Exp, scale=SCALE)
                            for jj in range(nb):
                                ii = i - nb + 1 + jj
                                mtt, hf = ii >> 1, ii & 1
                                nc.tensor.matmul(
                                    ps_ctx[hf][:],
                                    lhsT=vp_tiles[pt][:].rearrange(
                                        "q (m c) -> q m c", c=2 * VW)
                                    [:, mtt, hf * VW:(hf + 1) * VW],
                                    rhs=e_t[:, jj * 512:(jj + 1) * 512],
                                    start=(mtt == 0), stop=(mtt == MT_S - 1))
                    # ps rows: [0] = sum p, [32] = sum p*m*r, [64:128] = sum p*r*v~
                    # ctx = (ps[64:128] - wbar_v x ps[32]) / ps[0]
                    # one copy releases the PSUM bank for the next AV accum
                    for hf in range(2):
                        psc = tr.tile([P, 512], f32, name=f"psc_{pt}_{ch}_{hf}",
                                      tag="psc", bufs=2)
                        nc.vector.tensor_copy(psc[:], ps_ctx[hf][:])
                        rec = tr.tile([1, 512], bf16, name=f"rec_{pt}_{ch}_{hf}",
                                      tag="rec", bufs=2)
                        with nc.allow_low_precision(reason="softmax denom"):
                            nc.vector.reciprocal(rec[:], psc[0:1, :])
                        rbc = tr.tile([P, 512], bf16, name=f"rbc_{pt}_{ch}_{hf}",
                                      tag="recbc", bufs=2)
                        nc.gpsimd.partition_broadcast(rbc[:], rec[:])
                        pm_bc = tr.tile([P, 512], f32,
                                        name=f"pmbc_{pt}_{ch}_{hf}",
                                        tag="pmbc", bufs=2)
                        nc.gpsimd.partition_broadcast(pm_bc[:], psc[32:33, :])
                        corr = tr.tile([P, 512], bf16,
                                       name=f"corr_{pt}_{ch}_{hf}",
                                       tag="corr", bufs=2)
                        nc.vector.scalar_tensor_tensor(
                            corr[DK:P, :], pm_bc[DK:P, :],
                            aux_t[DK:P, 72 + 2 * pt + hf:73 + 2 * pt + hf],
                            psc[DK:P, :], ALU.mult, ALU.add)
                        if hf == 1:
                            nc.vector.tensor_mul(ctx_tiles[pt][DK:P, cs],
                                                 corr[DK:P, :], rbc[DK:P, :])
                        else:
                            shift = tr.tile([P, 512], bf16,
                                            name=f"sh_{pt}_{ch}", tag="shift",
                                            bufs=2)
                            nc.vector.tensor_mul(shift[DK:P, :], corr[DK:P, :],
                                                 rbc[DK:P, :])
                            nc.sync.dma_start(out=ctx_tiles[pt][0:DK, cs],
                                              in_=shift[DK:P, :])
                # W_o partials for this query chunk (overlaps next chunk)
                pwg = tr.tile([P, KT * 512], bf16, name=f"pwg_{ch}", tag="pwg",
                              bufs=2)
                for ot in range(KT):
                    ps = wops.tile([P, 512], f32, name=f"pso_{ch}_{ot}", tag="wo")
                    for pt in range(NPAIR):
                        nc.tensor.matmul(
                            ps[:],
                            lhsT=wog[:, pt * D + ot * P:pt * D + (ot + 1) * P],
                            rhs=ctx_tiles[pt][:, cs],
                            start=(pt == 0), stop=(pt == NPAIR - 1))
                    nc.vector.tensor_copy(pwg[:, ot * 512:(ot + 1) * 512], ps[:])
                nc.sync.dma_start(
                    out=part[ch * D:(ch + 1) * D, :].rearrange(
                        "(k p) c -> p k c", p=P),
                    in_=pwg[:].rearrange("p (k c) -> p k c", k=KT))
            wops.release()
            ctxps.release()
            scps.release()
            cp.release()
            vpp.release()
            kqp.release()

            nc.gpsimd.collective_compute(
                "ReduceScatter", ALU.add, ins=[part[:].opt()],
                outs=[rsout[:].opt()], replica_groups=groups)

            # ---- x2 = rs + b_o + x_own ----
            xsp = tc.alloc_tile_pool(name=f"xsp{it}", bufs=1)
            xsg = xsp.tile([P, KT * TOK], f32, name=f"xsg_{it}", tag="xsg")
            nc.sync.dma_start(
                out=xsg[:].rearrange("p (k c) -> p k c", k=KT),
                in_=xsT.rearrange("(k p) c -> p k c", p=P))
            xs_tiles = [xsg[:, kt * TOK:(kt + 1) * TOK] for kt in range(KT)]
            x2p = tc.alloc_tile_pool(name=f"x2p{it}", bufs=1)
            rsg = tr.tile([P, KT * TOK], bf16, name=f"rsg_{it}", tag="rsg",
                          bufs=1)
            nc.sync.dma_start(
                out=rsg[:].rearrange("p (k c) -> p k c", k=KT),
                in_=rsout[:].rearrange("(k p) c -> p k c", p=P))
            x2_tiles = []
            for ot in range(KT):
                x2 = x2p.tile([P, TOK], f32, name=f"x2_{it}_{ot}", tag=f"x2_{ot}")
                nc.vector.scalar_tensor_tensor(
                    x2[:], rsg[:, ot * TOK:(ot + 1) * TOK], bo_t[:, ot:ot + 1],
                    xs_tiles[ot][:], ALU.add, ALU.add)
                x2_tiles.append(x2)

            # ---- LN2 + FFN (own 512 tokens, bf16 weights) ----
            h2p = tc.alloc_tile_pool(name=f"h2p{it}", bufs=1)
            h2_tiles = _layer_norm(nc, tc, h2p, ones_b, x2_tiles, g2_t, be2_t,
                                   f"ln2_{it}", TOK)

            apool = tc.alloc_tile_pool(name=f"ap{it}", bufs=8)
            f1ps = tc.alloc_tile_pool(name=f"f1ps{it}", bufs=4, space="PSUM")
            f2ps = tc.alloc_tile_pool(name=f"f2ps{it}", bufs=4, space="PSUM")
            for mg in range(4):
                w1g = wp.tile([P, KT * D], bf16, name=f"w1g_{it}_{mg}", tag="w")
                nc.sync.dma_start(
                    out=w1g[:].rearrange("p (k c) -> p k c", k=KT),
                    in_=w1[:, mg * D:(mg + 1) * D].rearrange(
                        "(k p) c -> p k c", p=P))
                a_tiles = []
                for mt in range(KT):
                    m = mg * KT + mt
                    ps = f1ps.tile([P, TOK], f32, name=f"psf1_{m}", tag="f1")
                    for kt in range(KT):
                        nc.tensor.matmul(
                            ps[:],
                            lhsT=w1g[:, kt * D + mt * P:kt * D + (mt + 1) * P],
                            rhs=h2_tiles[kt][:], start=(kt == 0), stop=(kt == KT - 1))
                    at = apool.tile([P, TOK], bf16, name=f"a_{it}_{m}", tag="a")
                    nc.vector.tensor_scalar(at[:], ps[:], b1_t[:, m:m + 1], 0.0,
                                            ALU.add, ALU.max)
                    a_tiles.append(at)
                w2g = wp.tile([P, KT * D], bf16, name=f"w2g_{it}_{mg}", tag="w")
                nc.sync.dma_start(
                    out=w2g[:].rearrange("p (k c) -> p k c", k=KT),
                    in_=w2[mg * D:(mg + 1) * D, :].rearrange(
                        "(k p) c -> p k c", p=P))
                for mt in range(KT):
                    ps2 = f2ps.tile([P, TOK], f32, name=f"psf2_{mg}_{mt}", tag="f2")
                    for kt in range(KT):
                        nc.tensor.matmul(
                            ps2[:],
                            lhsT=w2g[:, kt * D + mt * P:kt * D + (mt + 1) * P],
                            rhs=a_tiles[kt][:], start=(kt == 0), stop=(kt == KT - 1))
                    nc.vector.tensor_add(x2_tiles[mt][:], x2_tiles[mt][:], ps2[:])
            f2ps.release()
            f1ps.release()
            apool.release()
            h2p.release()

            # ---- final bias + store (staged over the dead xs buffer) ----
            outg = xsg
            for mt in range(KT):
                nc.vector.tensor_scalar_add(outg[:, mt * TOK:(mt + 1) * TOK],
                                            x2_tiles[mt][:], b2_t[:, mt:mt + 1])
            nc.sync.dma_start(
                out=outT.rearrange("(k p) c -> p k c", p=P),
                in_=outg[:].rearrange("p (k c) -> p k c", k=KT))

            x2p.release()
            xsp.release()
            qkvw.release()

        dram.release()
        ep.release()
        wp.release()
        tr.release()
        sb.release()

    nc.compile()
    return nc


_NC_CACHE = {}


def _get_nc(n_iters: int = 1):
    if n_iters not in _NC_CACHE:
        _NC_CACHE[n_iters] = build(n_iters)
    return _NC_CACHE[n_iters]


def _pack_vec(v: np.ndarray, nt: int) -> np.ndarray:
    return np.ascontiguousarray(v.reshape(nt, P).T, dtype=np.float32)


def make_in_maps(inputs: dict) -> list:
    x = np.asarray(inputs["x"], dtype=np.float32)
    B, S_, D_ = x.shape
    assert (B, S_, D_) == (2, S, D)
    xf = x.reshape(B * S_, D_)

    g1 = np.asarray(inputs["ln1_g"], np.float32)
    b1n = np.asarray(inputs["ln1_b"], np.float32)
    wq_raw = np.asarray(inputs["W_q"], dtype=np.float32)
    wk_raw = np.asarray(inputs["W_k"], dtype=np.float32)
    wv_raw = np.asarray(inputs["W_v"], dtype=np.float32)
    wo = np.asarray(inputs["W_o"], dtype=np.float32)
    # fold LN1 gamma into the projections; beta terms become per-output
    # bias vectors (the V-side one collapses into b_o after W_o)
    wq = g1[:, None] * wq_raw
    wk = g1[:, None] * wk_raw
    wv = g1[:, None] * wv_raw
    qb_full = wq_raw.T @ b1n     # [D]
    kb_full = wk_raw.T @ b1n
    bo_adj = np.asarray(inputs["b_o"], np.float32) + wo.T @ (wv_raw.T @ b1n)

    shared = {
        "w1": np.ascontiguousarray(inputs["W1"]).astype(ml_dtypes.bfloat16),
        "w2": np.ascontiguousarray(inputs["W2"]).astype(ml_dtypes.bfloat16),
    }
    aux_common = np.concatenate([
        _pack_vec(bo_adj, KT),
        _pack_vec(np.asarray(inputs["b1"], np.float32), FT),
        _pack_vec(np.asarray(inputs["b2"], np.float32), KT),
        _pack_vec(np.asarray(inputs["ln2_g"], np.float32), KT),
        _pack_vec(np.asarray(inputs["ln2_b"], np.float32), KT),
    ], axis=1)   # [P, 64]
    xT_b = [np.ascontiguousarray(x[b].T).astype(ml_dtypes.bfloat16)
            for b in range(B)]

    in_maps = []
    for c in range(N_CORES):
        b, hc = c // GRP, c % GRP
        fs = slice(hc * HF, (hc + 1) * HF)
        wbar_q = wq[:, fs].sum(axis=0)   # [256]
        wbar_k = wk[:, fs].sum(axis=0)
        wbar_v = wv[:, fs].sum(axis=0)
        negwb = np.stack([-wbar_q[:P], -wbar_q[P:], -wbar_k[:P], -wbar_k[P:]],
                         axis=1).astype(np.float32)            # [128, 4]
        qkb = np.stack([qb_full[fs][:P], qb_full[fs][P:],
                        kb_full[fs][:P], kb_full[fs][P:]],
                       axis=1).astype(np.float32)              # [128, 4]
        negwv = np.zeros((P, 4), np.float32)
        negwv[DK:P, 0] = -wbar_v[0:DK]
        negwv[DK:P, 1] = -wbar_v[DK:2 * DK]
        negwv[DK:P, 2] = -wbar_v[2 * DK:3 * DK]
        negwv[DK:P, 3] = -wbar_v[3 * DK:]
        aux = np.concatenate([aux_common, negwb, qkb, negwv], axis=1)
        in_maps.append({
            "xT": xT_b[b],
            "xsT": np.ascontiguousarray(xf[c * TOK:(c + 1) * TOK, :].T),
            "wqh": np.ascontiguousarray(wq[:, fs]).astype(ml_dtypes.bfloat16),
            "wkh": np.ascontiguousarray(wk[:, fs]).astype(ml_dtypes.bfloat16),
            "wvh": np.ascontiguousarray(wv[:, fs]).astype(ml_dtypes.bfloat16),
            "woh": np.ascontiguousarray(wo[fs, :]).astype(ml_dtypes.bfloat16),
            "aux_v": np.ascontiguousarray(aux),
            **shared,
        })
    return in_maps


def run(inputs: dict, trace: bool = False):
    in_maps = make_in_maps(inputs)
    nc = _get_nc(1)
    res = run_bass_kernel_spmd(nc, in_maps, list(range(N_CORES)), trace=trace)
    out = np.empty((2 * S, D), dtype=np.float32)
    for c in range(N_CORES):
        out[c * TOK:(c + 1) * TOK, :] = res.results[c]["outT"].T
    return out.reshape(2, S, D), res


def kernel(**inputs) -> np.ndarray:
    out, _ = run(inputs, trace=False)
    return out


# revision 58
# speedup vs baseline: 1.3956x; 1.3956x over previous
"""Trainium2 Bass kernel for nn_EncoderBlock (dense transformer encoder block).

Strategy (8 NeuronCores, head-parallel attention, bf16 compute):
  - Cores 0-3 handle batch 0, cores 4-7 batch 1. Within a group each core
    owns 4 heads for attention (all 2048 tokens) and 512 tokens for the FFN.
  - LN1 is never materialized: Q/K/V are projected directly from raw x and
    the per-token affine is applied in the 256-dim projected space:
    q = r*(Wq'^T x - m*wbar_q) + qb  with Wq' = diag(ln1_g) Wq, column sums
    wbar and bias projections qb precomputed host-side. Only the per-token
    stats (mean, rstd rows via ones-matmul PE reduction) run on-chip, so
    projections start as soon as x chunks land.
  - Attention: transposed scores [k,q], exp on ACT (the phase's bottleneck,
    ~135us for 16.7M scores). The AV matmul uses a 128-row packed V block
    per (k-tile, head): [ones@0 | m*r@32 | r*V@64:128], so softmax sum and
    the V-side LN-mean correction come out of the same matmul at 32-aligned
    PSUM partitions. W_o partials are computed per query chunk, overlapped
    with the next chunk's attention.
  - One ReduceScatter (bf16, 1MB output) exchanges W_o partials - the only
    collective - yielding each core's own 512-token slice. The V-side LN
    beta term collapses into b_o host-side.
  - Residual, LN2 (classic, f32 stats matmuls while PE is idle), FFN
    token-parallel with W1/W2 in bf16 streamed as one batched DMA per
    1024-column group; FFN phase runs at 100% PE.
  - All matmuls bf16 (1 PE cycle/row). DMAs are batched via rearranged
    access patterns to keep HWDGE descriptor generation off the critical
    path. TimelineSim: 447us/iter (baseline restructured from 863us).
"""

import numpy as np

import ml_dtypes

import concourse.bass as bass
import concourse.mybir as mybir
import concourse.tile as tile
from concourse import bacc
from concourse.bass_utils import run_bass_kernel_spmd

N_CORES = 8
GRP = 4          # cores per batch group
P = 128
TOK = 512        # own tokens per core (FFN / output slice)
S = 2048         # sequence length (tokens per batch)
D = 1024
KT = D // P      # 8 feature tiles
HF = 256         # head features per core (4 heads x 64)
NPAIR = 2        # head pairs per core
DK = 64
DK1 = DK + 1
VW = 128       # AV block: [ones@0 | m*r@32 | V@64:128]
F = 4096
FT = F // P      # 32 ffn tiles
EPS = 1e-6
SCALE = 0.125    # 1/sqrt(DK)
MT_S = S // P    # 16 k-token tiles per batch
QC = S // TOK    # 4 query chunks of 512
NBLK = 2         # score blocks per psum tile / exp call

f32 = mybir.dt.float32
f32r = mybir.dt.float32r
bf16 = mybir.dt.bfloat16
ALU = mybir.AluOpType
ACT = mybir.ActivationFunctionType


def tf32_round(x: np.ndarray) -> np.ndarray:
    u = np.ascontiguousarray(x, dtype=np.float32).view(np.uint32)
    lsb = (u >> np.uint32(13)) & np.uint32(1)
    u = u + np.uint32(0x0FFF) + lsb
    u = u & np.uint32(0xFFFFE000)
    return u.view(np.float32)


def _layer_norm(nc, tc, hpool, ones_b, x_tiles, g_t, b_t, tag, width,
                in_bf16=False, inplace=False):
    """Feature-major layernorm over KT [128, width] tiles, bf16 output.

    Per-token (free-dim) stats via ones-matmul partition reduction on PE,
    chunked by 512 columns (PSUM bank limit). r = 1/sqrt on ACT.Sqrt + DVE
    reciprocal (avoids Ln/Exp table swaps). If inplace, the normalized
    output overwrites x_tiles (requires in_bf16).
    """
    CH = width // 512
    rows = tc.alloc_tile_pool(name=f"lnrow_{tag}", bufs=1)
    ltr = tc.alloc_tile_pool(name=f"lntr_{tag}", bufs=2)
    lnps = tc.alloc_tile_pool(name=f"lnps_{tag}", bufs=2, space="PSUM")

    inv_n = 1.0 / D
    h_tiles = []
    if not inplace:
        for kt in range(KT):
            h_tiles.append(hpool.tile([P, width], bf16,
                                      name=f"h_{tag}_{kt}", tag=f"h_{kt}"))
    mean = rows.tile([1, width], f32, name=f"mean_{tag}", tag="mean")
    var = rows.tile([1, width], f32, name=f"var_{tag}", tag="var")
    for ch in range(CH):
        cs = slice(ch * 512, (ch + 1) * 512)
        ps_sum = lnps.tile([1, 512], f32, name=f"pssum_{tag}_{ch}", tag="pssum")
        ps_sq = lnps.tile([1, 512], f32, name=f"pssq_{tag}_{ch}", tag="pssq")
        for kt in range(KT):
            sq = ltr.tile([P, 512], bf16, name=f"sq_{tag}_{ch}_{kt}",
                          tag="xrsq", bufs=3)
            nc.vector.tensor_mul(sq[:], x_tiles[kt][:, cs], x_tiles[kt][:, cs])
            if in_bf16:
                xr_ap = x_tiles[kt][:, cs]
            else:
                xr = ltr.tile([P, 512], bf16, name=f"xr_{tag}_{ch}_{kt}",
                              tag="xrsq", bufs=3)
                nc.scalar.copy(xr[:], x_tiles[kt][:, cs])
                xr_ap = xr[:]
            nc.tensor.matmul(ps_sum[:], lhsT=ones_b[:], rhs=xr_ap,
                             start=(kt == 0), stop=(kt == KT - 1))
            nc.tensor.matmul(ps_sq[:], lhsT=ones_b[:], rhs=sq[:],
                             start=(kt == 0), stop=(kt == KT - 1))
        msq = rows.tile([1, 512], f32, name=f"msq_{tag}_{ch}", tag="msqlnv",
                        bufs=2)
        nc.vector.tensor_scalar_mul(mean[:, cs], ps_sum[:], inv_n)
        nc.vector.tensor_scalar_mul(var[:, cs], ps_sq[:], inv_n)
        nc.vector.tensor_mul(msq[:], mean[:, cs], mean[:, cs])
        nc.vector.tensor_sub(var[:, cs], var[:, cs], msq[:])
        nc.vector.tensor_scalar_add(var[:, cs], var[:, cs], EPS)

    # r = 1/sqrt(var+eps): ACT.Sqrt then DVE reciprocal
    sd = rows.tile([1, width], f32, name=f"sd_{tag}", tag="sd")
    nc.scalar.activation(sd[:], var[:], ACT.Sqrt)
    r_row = rows.tile([1, width], bf16, name=f"r_{tag}", tag="r")
    with nc.allow_low_precision(reason="bf16 layernorm scale is plenty"):
        nc.vector.reciprocal(r_row[:], sd[:])
    mr_row = rows.tile([1, width], bf16, name=f"mr_{tag}", tag="mr")
    nc.vector.tensor_mul(mr_row[:], mean[:], r_row[:])

    for ch in range(CH):
        cs = slice(ch * 512, (ch + 1) * 512)
        r_bc = rows.tile([P, 512], bf16, name=f"rbc_{tag}_{ch}", tag="rbc",
                         bufs=2)
        mr_bc = rows.tile([P, 512], bf16, name=f"mrbc_{tag}_{ch}", tag="mrbc",
                          bufs=2)
        nc.gpsimd.partition_broadcast(r_bc[:], r_row[:, cs])
        nc.gpsimd.partition_broadcast(mr_bc[:], mr_row[:, cs])
        for kt in range(KT):
            t1 = ltr.tile([P, 512], bf16, name=f"t1_{tag}_{ch}_{kt}",
                          tag="lnt1", bufs=3)
            nc.vector.tensor_mul(t1[:], x_tiles[kt][:, cs], r_bc[:])
            nc.vector.tensor_sub(t1[:], t1[:], mr_bc[:])
            h = x_tiles[kt] if inplace else h_tiles[kt]
            nc.vector.tensor_scalar(h[:, cs], t1[:], g_t[:, kt:kt + 1],
                                    b_t[:, kt:kt + 1], ALU.mult, ALU.add)
    if inplace:
        h_tiles = x_tiles
    lnps.release()
    ltr.release()
    rows.release()
    return h_tiles


def build(n_iters: int = 1):
    nc = bacc.Bacc("TRN2", target_bir_lowering=False, debug=False,
                   num_devices=N_CORES)

    xT = nc.dram_tensor("xT", [D, S], bf16, kind="ExternalInput").ap()
    xsT = nc.dram_tensor("xsT", [D, TOK], f32, kind="ExternalInput").ap()
    wqh = nc.dram_tensor("wqh", [D, HF], bf16, kind="ExternalInput").ap()
    wkh = nc.dram_tensor("wkh", [D, HF], bf16, kind="ExternalInput").ap()
    wvh = nc.dram_tensor("wvh", [D, HF], bf16, kind="ExternalInput").ap()
    woh = nc.dram_tensor("woh", [HF, D], bf16, kind="ExternalInput").ap()
    w1 = nc.dram_tensor("w1", [D, F], bf16, kind="ExternalInput").ap()
    w2 = nc.dram_tensor("w2", [F, D], bf16, kind="ExternalInput").ap()
    aux_v = nc.dram_tensor("aux_v", [P, 76], f32, kind="ExternalInput").ap()

    outT = nc.dram_tensor("outT", [D, TOK], f32, kind="ExternalOutput").ap()

    groups = [[0, 1, 2, 3], [4, 5, 6, 7]]

    with tile.TileContext(nc) as tc:
        sb = tc.alloc_tile_pool(name="sb", bufs=1)
        tr = tc.alloc_tile_pool(name="tr", bufs=3)
        wp = tc.alloc_tile_pool(name="wp", bufs=2)
        ep = tc.alloc_tile_pool(name="ep", bufs=3)
        dram = tc.alloc_tile_pool(name="dram", bufs=1, space="DRAM")

        # ---- constants / small inputs ----
        ones_f = sb.tile([P, 32], bf16, name="ones_f", tag="ones_f")
        nc.vector.memset(ones_f[:], 1.0)
        ones_b = sb.tile([P, 1], bf16, name="ones_b", tag="ones_b")
        nc.vector.tensor_copy(ones_b[:], ones_f[:, 0:1])
        ones_g = sb.tile([P, 1], f32, name="ones_g", tag="ones_g")
        nc.vector.memset(ones_g[:], 1.0)
        aux_t = sb.tile([P, 76], f32, name="aux_t", tag="aux_t")
        nc.sync.dma_start(out=aux_t[:], in_=aux_v)
        bo_t = aux_t[:, 0:8]
        b1_t = aux_t[:, 8:40]
        b2_t = aux_t[:, 40:48]
        g2_t = aux_t[:, 48:56]
        be2_t = aux_t[:, 56:64]

        part = dram.tile([GRP * D, TOK], bf16, name="part", tag="part")
        rsout = dram.tile([D, TOK], bf16, name="rsout", tag="rsout")
        rrow_d = dram.tile([1, S], f32, name="rrow_d", tag="rrow_d")
        mrrow_d = dram.tile([1, S], f32, name="mrrow_d", tag="mrrow_d")

        for it in range(n_iters):
            # ---- load QKV weight slices (full x comes last in the stack) ----
            qkvw = tc.alloc_tile_pool(name=f"qkvw{it}", bufs=1)
            wgt = {}
            for w_dram, nm in [(wkh, "k"), (wvh, "v"), (wqh, "q")]:
                t = qkvw.tile([P, KT * HF], bf16, name=f"w{nm}g_{it}",
                              tag=f"w{nm}g")
                nc.sync.dma_start(
                    out=t[:].rearrange("p (k c) -> p k c", k=KT),
                    in_=w_dram.rearrange("(k p) c -> p k c", p=P))
                wgt[nm] = t
            wog = qkvw.tile([P, NPAIR * D], bf16, name=f"wog_{it}", tag="wog")
            nc.sync.dma_start(
                out=wog[:].rearrange("p (k c) -> p k c", k=NPAIR),
                in_=woh.rearrange("(k p) c -> p k c", p=P))

            # attention destination pools sit below xp so xp can release first
            kqp = tc.alloc_tile_pool(name=f"kqp{it}", bufs=1)
            k_tiles, q_tiles = [], []
            for nm, lst in [("k", k_tiles), ("q", q_tiles)]:
                for pt in range(NPAIR):
                    lst.append(kqp.tile([P, S], bf16, name=f"{nm}T_{it}_{pt}",
                                        tag=f"{nm}T_{pt}"))
            vpp = tc.alloc_tile_pool(name=f"vpp{it}", bufs=1)
            vp_tiles = []
            for pt in range(NPAIR):
                vp = vpp.tile([P, MT_S * 2 * VW], bf16, name=f"vp_{it}_{pt}",
                              tag=f"vp_{pt}")
                nc.vector.memset(vp[:], 0.0)
                vp_tiles.append(vp)
            cp = tc.alloc_tile_pool(name=f"cp{it}", bufs=1)
            ctx_tiles = []
            for pt in range(NPAIR):
                ctx_tiles.append(cp.tile([P, S], bf16, name=f"ctx_{it}_{pt}",
                                         tag=f"ctx_{pt}"))

            xp = tc.alloc_tile_pool(name=f"xp{it}", bufs=1)
            xg = xp.tile([P, KT * S], bf16, name=f"xg_{it}", tag="xg")
            x_tiles = [xg[:, kt * S:(kt + 1) * S] for kt in range(KT)]
            # two column-chunked loads so LN1 stats can start early
            for ch in range(2):
                cs = slice(ch * 1024, (ch + 1) * 1024)
                nc.sync.dma_start(
                    out=xg[:].rearrange("p (k c) -> p k c", k=KT)[:, :, cs],
                    in_=xT[:, cs].rearrange("(k p) c -> p k c", p=P))

            # ---- LN1 stats on raw x (projections absorb the normalization:
            #      q = r*(Wq'^T x - m*wbar_q) + qb, with Wq' = diag(g)Wq and
            #      wbar/qb precomputed host-side) ----
            lnp = tc.alloc_tile_pool(name=f"lnp{it}", bufs=1)
            ltr = tc.alloc_tile_pool(name=f"lntr{it}", bufs=2)
            lnps = tc.alloc_tile_pool(name=f"lnps{it}", bufs=2, space="PSUM")
            mean = lnp.tile([1, S], f32, name=f"mean_{it}", tag="mean")
            var = lnp.tile([1, S], f32, name=f"var_{it}", tag="var")
            for ch in range(QC):
                cs = slice(ch * 512, (ch + 1) * 512)
                ps_sum = lnps.tile([1, 512], f32, name=f"pssum_{ch}", tag="pssum")
                ps_sq = lnps.tile([1, 512], f32, name=f"pssq_{ch}", tag="pssq")
                for kt in range(KT):
                    sq = ltr.tile([P, 512], bf16, name=f"sq_{ch}_{kt}",
                                  tag="xrsq", bufs=3)
                    nc.scalar.square(sq[:], x_tiles[kt][:, cs])
                    nc.tensor.matmul(ps_sum[:], lhsT=ones_b[:],
                                     rhs=x_tiles[kt][:, cs],
                                     start=(kt == 0), stop=(kt == KT - 1))
                    nc.tensor.matmul(ps_sq[:], lhsT=ones_b[:], rhs=sq[:],
                                     start=(kt == 0), stop=(kt == KT - 1))
                msq = lnp.tile([1, 512], f32, name=f"msq_{ch}", tag="msq",
                               bufs=1)
                nc.vector.tensor_scalar_mul(mean[:, cs], ps_sum[:], 1.0 / D)
                nc.vector.tensor_scalar_mul(var[:, cs], ps_sq[:], 1.0 / D)
                nc.vector.tensor_mul(msq[:], mean[:, cs], mean[:, cs])
                nc.vector.tensor_sub(var[:, cs], var[:, cs], msq[:])
                nc.vector.tensor_scalar_add(var[:, cs], var[:, cs], EPS)
            m_row = lnp.tile([1, S], bf16, name=f"m_{it}", tag="m")
            nc.vector.tensor_copy(m_row[:], mean[:])
            # var -> sqrt -> reciprocal in place; mean -> mean*r in place
            nc.scalar.activation(var[:], var[:], ACT.Sqrt)
            nc.vector.reciprocal(var[:], var[:])
            r_row = lnp.tile([1, S], bf16, name=f"r_{it}", tag="r")
            nc.vector.tensor_copy(r_row[:], var[:])
            nc.vector.tensor_mul(mean[:], mean[:], var[:])
            # token-major r and m*r columns for the V path
            rcol = lnp.tile([P, MT_S], f32, name=f"rcol_{it}", tag="rcol")
            mrcol = lnp.tile([P, MT_S], f32, name=f"mrcol_{it}", tag="mrcol")
            nc.sync.dma_start(out=rrow_d[:], in_=var[:])
            nc.sync.dma_start(out=mrrow_d[:], in_=mean[:])
            nc.sync.dma_start(
                out=rcol[:], in_=rrow_d[:].rearrange("o (m p) -> (o p) m", p=P))
            nc.sync.dma_start(
                out=mrcol[:],
                in_=mrrow_d[:].rearrange("o (m p) -> (o p) m", p=P))
            lnps.release()

            # ---- K^T / Q^T projections on raw x + LN correction ----
            qkvps = tc.alloc_tile_pool(name=f"qkvps{it}", bufs=4, space="PSUM")
            mbc_tiles, rbc_tiles = [], []
            for ch in range(QC):
                cs = slice(ch * 512, (ch + 1) * 512)
                m_bc = lnp.tile([P, 512], bf16, name=f"mbc_{ch}", tag=f"mbc_{ch}")
                r_bc = lnp.tile([P, 512], bf16, name=f"rbc_{ch}", tag=f"rbc_{ch}")
                nc.gpsimd.partition_broadcast(m_bc[:], m_row[:, cs])
                nc.gpsimd.partition_broadcast(r_bc[:], r_row[:, cs])
                mbc_tiles.append(m_bc)
                rbc_tiles.append(r_bc)
            for dsts, nm, wb_i, bb_i in [
                    (k_tiles, "k", 2, 2), (q_tiles, "q", 0, 0)]:
                wg = wgt[nm]
                for pt in range(NPAIR):
                    dst = dsts[pt]
                    for ch in range(QC):
                        cs = slice(ch * 512, (ch + 1) * 512)
                        ps = qkvps.tile([P, 512], f32, name=f"ps{nm}_{pt}_{ch}",
                                        tag="qkv")
                        for kt in range(KT):
                            nc.tensor.matmul(
                                ps[:],
                                lhsT=wg[:, kt * HF + pt * P:kt * HF + (pt + 1) * P],
                                rhs=x_tiles[kt][:, cs], start=(kt == 0),
                                stop=(kt == KT - 1))
                        u = ltr.tile([P, 512], bf16, name=f"u{nm}_{pt}_{ch}",
                                     tag="u", bufs=3)
                        nc.vector.scalar_tensor_tensor(
                            u[:], mbc_tiles[ch][:],
                            aux_t[:, 64 + wb_i + pt:65 + wb_i + pt], ps[:],
                            ALU.mult, ALU.add)
                        nc.vector.tensor_mul(dst[:, cs], u[:], rbc_tiles[ch][:])
                        nc.vector.tensor_scalar_add(
                            dst[:, cs], dst[:, cs],
                            aux_t[:, 68 + bb_i + pt:69 + bb_i + pt])

            # ---- V projection (token-major, [r*V | m*r | 1] packing) ----
            for pt in range(NPAIR):
                vpv = vp_tiles[pt][:].rearrange("q (k c) -> q k c", c=VW)
                nc.vector.tensor_copy(vpv[:, :, 0:1].squeeze(2), ones_f[:])
                for hf in range(2):
                    nc.vector.tensor_copy(
                        vp_tiles[pt][:].rearrange("q (m h c) -> q m h c", h=2,
                                                  c=VW)
                        [:, :, hf, 32:33].squeeze(2), mrcol[:])
            for mt in range(MT_S):
                ps = qkvps.tile([P, HF], f32, name=f"psv_{mt}", tag="qkv")
                for kt in range(KT):
                    nc.tensor.matmul(
                        ps[:], lhsT=x_tiles[kt][:, mt * P:(mt + 1) * P],
                        rhs=wgt["v"][:, kt * HF:(kt + 1) * HF],
                        start=(kt == 0), stop=(kt == KT - 1))
                for pt in range(NPAIR):
                    for hf in range(2):
                        dst = vp_tiles[pt][:].rearrange(
                            "q (m c) -> q m c", c=2 * VW)[:, mt,
                            hf * VW + DK:hf * VW + P]
                        nc.vector.tensor_scalar_mul(
                            dst, ps[:, pt * P + hf * DK:pt * P + (hf + 1) * DK],
                            rcol[:, mt:mt + 1])
            qkvps.release()
            ltr.release()
            lnp.release()
            xp.release()


            # ---- attention (4 query chunks x 2 head pairs), W_o fused ----
            scps = tc.alloc_tile_pool(name=f"scps{it}", bufs=2, space="PSUM")
            ctxps = tc.alloc_tile_pool(name=f"ctxps{it}", bufs=2, space="PSUM")
            wops = tc.alloc_tile_pool(name=f"wops{it}", bufs=2, space="PSUM")
            for ch in range(QC):
                cs = slice(ch * 512, (ch + 1) * 512)
                for pt in range(NPAIR):
                    ps_ctx = [ctxps.tile([P, 512], f32,
                                         name=f"psctx_{pt}_{ch}_{hh}",
                                         tag="psctx") for hh in range(2)]
                    ps_sc = None
                    e_t = None
                    nb = 0
                    for i in range(2 * MT_S):
                        mt, half = i >> 1, i & 1
                        j = i % NBLK
                        if j == 0:
                            nb = min(NBLK, 2 * MT_S - i)
                            ps_sc = scps.tile([P, NBLK * 512], f32,
                                              name=f"pssc_{pt}_{ch}_{i}",
                                              tag="pssc")
                            e_t = ep.tile([P, NBLK * 512], bf16,
                                          name=f"e_{pt}_{ch}_{i}", tag="e")
                        nc.tensor.matmul(
                            ps_sc[:, j * 512:(j + 1) * 512],
                            lhsT=k_tiles[pt][half * DK:(half + 1) * DK,
                                             mt * P:(mt + 1) * P],
                            rhs=q_tiles[pt][half * DK:(half + 1) * DK, cs],
                            start=True, stop=True)
                        if j == nb - 1:
                            nc.scalar.activation(e_t[:, 0:nb * 512],
                                                 ps_sc[:, 0:nb * 512],
                                                 ACT.Exp, scale=SCALE)
                            for jj in range(nb):
                                ii = i - nb + 1 + jj
                                mtt, hf = ii >> 1, ii & 1
                                nc.tensor.matmul(
                                    ps_ctx[hf][:],
                                    lhsT=vp_tiles[pt][:].rearrange(
                                        "q (m c) -> q m c", c=2 * VW)
                                    [:, mtt, hf * VW:(hf + 1) * VW],
                                    rhs=e_t[:, jj * 512:(jj + 1) * 512],
                                    start=(mtt == 0), stop=(mtt == MT_S - 1))
                    # ps rows: [0] = sum p, [32] = sum p*m*r, [64:128] = sum p*r*v~
                    # ctx = (ps[64:128] - wbar_v x ps[32]) / ps[0]
                    # one copy releases the PSUM bank for the next AV accum
                    for hf in range(2):
                        psc = tr.tile([P, 512], f32, name=f"psc_{pt}_{ch}_{hf}",
                                      tag="psc", bufs=2)
                        nc.vector.tensor_copy(psc[:], ps_ctx[hf][:])
                        rec = tr.tile([1, 512], bf16, name=f"rec_{pt}_{ch}_{hf}",
                                      tag="rec", bufs=2)
                        with nc.allow_low_precision(reason="softmax denom"):
                            nc.vector.reciprocal(rec[:], psc[0:1, :])
                        rbc = tr.tile([P, 512], bf16, name=f"rbc_{pt}_{ch}_{hf}",
                                      tag="recbc", bufs=2)
                        nc.gpsimd.partition_broadcast(rbc[:], rec[:])
                        pm_bc = tr.tile([P, 512], f32,
                                        name=f"pmbc_{pt}_{ch}_{hf}",
                                        tag="pmbc", bufs=2)
                        nc.gpsimd.partition_broadcast(pm_bc[:], psc[32:33, :])
                        corr = tr.tile([P, 512], bf16,
                                       name=f"corr_{pt}_{ch}_{hf}",
                                       tag="corr", bufs=2)
                        nc.vector.scalar_tensor_tensor(
                            corr[DK:P, :], pm_bc[DK:P, :],
                            aux_t[DK:P, 72 + 2 * pt + hf:73 + 2 * pt + hf],
                            psc[DK:P, :], ALU.mult, ALU.add)
                        if hf == 1:
                            nc.vector.tensor_mul(ctx_tiles[pt][DK:P, cs],
                                                 corr[DK:P, :], rbc[DK:P, :])
                        else:
                            shift = tr.tile([P, 512], bf16,
                                            name=f"sh_{pt}_{ch}", tag="shift",
                                            bufs=2)
                            nc.vector.tensor_mul(shift[DK:P, :], corr[DK:P, :],
                                                 rbc[DK:P, :])
                            nc.sync.dma_start(out=ctx_tiles[pt][0:DK, cs],
                                              in_=shift[DK:P, :])
                # W_o partials for this query chunk (overlaps next chunk)
                pwg = tr.tile([P, KT * 512], bf16, name=f"pwg_{ch}", tag="pwg",
                              bufs=2)
                for ot in range(KT):
                    ps = wops.tile([P, 512], f32, name=f"pso_{ch}_{ot}", tag="wo")
                    for pt in range(NPAIR):
                        nc.tensor.matmul(
                            ps[:],
                            lhsT=wog[:, pt * D + ot * P:pt * D + (ot + 1) * P],
                            rhs=ctx_tiles[pt][:, cs],
                            start=(pt == 0), stop=(pt == NPAIR - 1))
                    nc.vector.tensor_copy(pwg[:, ot * 512:(ot + 1) * 512], ps[:])
                nc.sync.dma_start(
                    out=part[ch * D:(ch + 1) * D, :].rearrange(
                        "(k p) c -> p k c", p=P),
                    in_=pwg[:].rearrange("p (k c) -> p k c", k=KT))
            wops.release()
            ctxps.release()
            scps.release()
            cp.release()
            vpp.release()
            kqp.release()

            nc.gpsimd.collective_compute(
                "ReduceScatter", ALU.add, ins=[part[:].opt()],
                outs=[rsout[:].opt()], replica_groups=groups)

            # ---- x2 = rs + b_o + x_own ----
            xsp = tc.alloc_tile_pool(name=f"xsp{it}", bufs=1)
            xsg = xsp.tile([P, KT * TOK], f32, name=f"xsg_{it}", tag="xsg")
            nc.sync.dma_start(
                out=xsg[:].rearrange("p (k c) -> p k c", k=KT),
                in_=xsT.rearrange("(k p) c -> p k c", p=P))
            xs_tiles = [xsg[:, kt * TOK:(kt + 1) * TOK] for kt in range(KT)]
            x2p = tc.alloc_tile_pool(name=f"x2p{it}", bufs=1)
            rsg = tr.tile([P, KT * TOK], bf16, name=f"rsg_{it}", tag="rsg",
                          bufs=1)
            nc.sync.dma_start(
                out=rsg[:].rearrange("p (k c) -> p k c", k=KT),
                in_=rsout[:].rearrange("(k p) c -> p k c", p=P))
            x2_tiles = []
            for ot in range(KT):
                x2 = x2p.tile([P, TOK], f32, name=f"x2_{it}_{ot}", tag=f"x2_{ot}")
                nc.vector.scalar_tensor_tensor(
                    x2[:], rsg[:, ot * TOK:(ot + 1) * TOK], bo_t[:, ot:ot + 1],
                    xs_tiles[ot][:], ALU.add, ALU.add)
                x2_tiles.append(x2)

            # ---- LN2 + FFN (own 512 tokens, bf16 weights) ----
            h2p = tc.alloc_tile_pool(name=f"h2p{it}", bufs=1)
            h2_tiles = _layer_norm(nc, tc, h2p, ones_b, x2_tiles, g2_t, be2_t,
                                   f"ln2_{it}", TOK)

            apool = tc.alloc_tile_pool(name=f"ap{it}", bufs=8)
            f1ps = tc.alloc_tile_pool(name=f"f1ps{it}", bufs=4, space="PSUM")
            f2ps = tc.alloc_tile_pool(name=f"f2ps{it}", bufs=4, space="PSUM")
            for mg in range(4):
                w1g = wp.tile([P, KT * D], bf16, name=f"w1g_{it}_{mg}", tag="w")
                nc.sync.dma_start(
                    out=w1g[:].rearrange("p (k c) -> p k c", k=KT),
                    in_=w1[:, mg * D:(mg + 1) * D].rearrange(
                        "(k p) c -> p k c", p=P))
                a_tiles = []
                for mt in range(KT):
                    m = mg * KT + mt
                    ps = f1ps.tile([P, TOK], f32, name=f"psf1_{m}", tag="f1")
                    for kt in range(KT):
                        nc.tensor.matmul(
                            ps[:],
                            lhsT=w1g[:, kt * D + mt * P:kt * D + (mt + 1) * P],
                            rhs=h2_tiles[kt][:], start=(kt == 0), stop=(kt == KT - 1))
                    at = apool.tile([P, TOK], bf16, name=f"a_{it}_{m}", tag="a")
                    nc.vector.tensor_scalar(at[:], ps[:], b1_t[:, m:m + 1], 0.0,
                                            ALU.add, ALU.max)
                    a_tiles.append(at)
                w2g = wp.tile([P, KT * D], bf16, name=f"w2g_{it}_{mg}", tag="w")
                nc.sync.dma_start(
                    out=w2g[:].rearrange("p (k c) -> p k c", k=KT),
                    in_=w2[mg * D:(mg + 1) * D, :].rearrange(
                        "(k p) c -> p k c", p=P))
                for mt in range(KT):
                    ps2 = f2ps.tile([P, TOK], f32, name=f"psf2_{mg}_{mt}", tag="f2")
                    for kt in range(KT):
                        nc.tensor.matmul(
                            ps2[:],
                            lhsT=w2g[:, kt * D + mt * P:kt * D + (mt + 1) * P],
                            rhs=a_tiles[kt][:], start=(kt == 0), stop=(kt == KT - 1))
                    nc.vector.tensor_add(x2_tiles[mt][:], x2_tiles[mt][:], ps2[:])
            f2ps.release()
            f1ps.release()
            apool.release()
            h2p.release()

            # ---- final bias + store (staged over the dead xs buffer) ----
            outg = xsg
            for mt in range(KT):
                nc.vector.tensor_scalar_add(outg[:, mt * TOK:(mt + 1) * TOK],
                                            x2_tiles[mt][:], b2_t[:, mt:mt + 1])
            nc.sync.dma_start(
                out=outT.rearrange("(k p) c -> p k c", p=P),
                in_=outg[:].rearrange("p (k c) -> p k c", k=KT))

            x2p.release()
            xsp.release()
            qkvw.release()

        dram.release()
        ep.release()
        wp.release()
        tr.release()
        sb.release()

    nc.compile()
    return nc


_NC_CACHE = {}


def _get_nc(n_iters: int = 1):
    if n_iters not in _NC_CACHE:
        _NC_CACHE[n_iters] = build(n_iters)
    return _NC_CACHE[n_iters]


def _pack_vec(v: np.ndarray, nt: int) -> np.ndarray:
    return np.ascontiguousarray(v.reshape(nt, P).T, dtype=np.float32)


def make_in_maps(inputs: dict) -> list:
    x = np.asarray(inputs["x"], dtype=np.float32)
    B, S_, D_ = x.shape
    assert (B, S_, D_) == (2, S, D)
    xf = x.reshape(B * S_, D_)

    g1 = np.asarray(inputs["ln1_g"], np.float32)
    b1n = np.asarray(inputs["ln1_b"], np.float32)
    wq_raw = np.asarray(inputs["W_q"], dtype=np.float32)
    wk_raw = np.asarray(inputs["W_k"], dtype=np.float32)
    wv_raw = np.asarray(inputs["W_v"], dtype=np.float32)
    wo = np.asarray(inputs["W_o"], dtype=np.float32)
    # fold LN1 gamma into the projections; beta terms become per-output
    # bias vectors (the V-side one collapses into b_o after W_o)
    wq = g1[:, None] * wq_raw
    wk = g1[:, None] * wk_raw
    wv = g1[:, None] * wv_raw
    qb_full = wq_raw.T @ b1n     # [D]
    kb_full = wk_raw.T @ b1n
    bo_adj = np.asarray(inputs["b_o"], np.float32) + wo.T @ (wv_raw.T @ b1n)

    shared = {
        "w1": np.ascontiguousarray(inputs["W1"]).astype(ml_dtypes.bfloat16),
        "w2": np.ascontiguousarray(inputs["W2"]).astype(ml_dtypes.bfloat16),
    }
    aux_common = np.concatenate([
        _pack_vec(bo_adj, KT),
        _pack_vec(np.asarray(inputs["b1"], np.float32), FT),
        _pack_vec(np.asarray(inputs["b2"], np.float32), KT),
        _pack_vec(np.asarray(inputs["ln2_g"], np.float32), KT),
        _pack_vec(np.asarray(inputs["ln2_b"], np.float32), KT),
    ], axis=1)   # [P, 64]
    xT_b = [np.ascontiguousarray(x[b].T).astype(ml_dtypes.bfloat16)
            for b in range(B)]

    in_maps = []
    for c in range(N_CORES):
        b, hc = c // GRP, c % GRP
        fs = slice(hc * HF, (hc + 1) * HF)
        wbar_q = wq[:, fs].sum(axis=0)   # [256]
        wbar_k = wk[:, fs].sum(axis=0)
        wbar_v = wv[:, fs].sum(axis=0)
        negwb = np.stack([-wbar_q[:P], -wbar_q[P:], -wbar_k[:P], -wbar_k[P:]],
                         axis=1).astype(np.float32)            # [128, 4]
        qkb = np.stack([qb_full[fs][:P], qb_full[fs][P:],
                        kb_full[fs][:P], kb_full[fs][P:]],
                       axis=1).astype(np.float32)              # [128, 4]
        negwv = np.zeros((P, 4), np.float32)
        negwv[DK:P, 0] = -wbar_v[0:DK]
        negwv[DK:P, 1] = -wbar_v[DK:2 * DK]
        negwv[DK:P, 2] = -wbar_v[2 * DK:3 * DK]
        negwv[DK:P, 3] = -wbar_v[3 * DK:]
        aux = np.concatenate([aux_common, negwb, qkb, negwv], axis=1)
        in_maps.append({
            "xT": xT_b[b],
            "xsT": np.ascontiguousarray(xf[c * TOK:(c + 1) * TOK, :].T),
            "wqh": np.ascontiguousarray(wq[:, fs]).astype(ml_dtypes.bfloat16),
            "wkh": np.ascontiguousarray(wk[:, fs]).astype(ml_dtypes.bfloat16),
            "wvh": np.ascontiguousarray(wv[:, fs]).astype(ml_dtypes.bfloat16),
            "woh": np.ascontiguousarray(wo[fs, :]).astype(ml_dtypes.bfloat16),
            "aux_v": np.ascontiguousarray(aux),
            **shared,
        })
    return in_maps


def run(inputs: dict, trace: bool = False):
    in_maps = make_in_maps(inputs)
    nc = _get_nc(1)
    res = run_bass_kernel_spmd(nc, in_maps, list(range(N_CORES)), trace=trace)
    out = np.empty((2 * S, D), dtype=np.float32)
    for c in range(N_CORES):
        out[c * TOK:(c + 1) * TOK, :] = res.results[c]["outT"].T
    return out.reshape(2, S, D), res


def kernel(**inputs) -> np.ndarray:
    out, _ = run(inputs, trace=False)
    return out
